# revision 1
# baseline (speedup 1.0000x reference)
import numpy as np
import jax
import jax.numpy as jnp

# Hardcoded problem shapes (nn_GAT: B batches, N nodes, D in-feats, H heads,
# HID per-head hidden, EN output feats).
B, N, D, H, HID, EN = 2, 4096, 8, 4, 32, 8
ALPHA = 0.2
NEG_INF = -9e15
NCORES = 8
NR = N // (NCORES // B)  # rows per shard in layer 2 (1024)


def _layer1_shard(x_b, adj_b, W_h, a_h):
    # One (batch, head) pair: full NxN attention for a single head.
    wh = x_b @ W_h                                   # [N, HID]
    wh1 = wh @ a_h[:HID, :]                          # [N, 1]
    wh2 = wh @ a_h[HID:, :]                          # [N, 1]
    e = jax.nn.leaky_relu(wh1 + wh2.T, ALPHA)        # [N, N]
    nz = adj_b != 0
    scores = jnp.where(nz, e, NEG_INF) * jnp.where(nz, adj_b, 1.0)
    att = jax.nn.softmax(scores, axis=-1)
    return att @ wh                                  # [N, HID]


def _layer2_shard(h_full, h_rows, adj_rows, W_last, a_last):
    # One (batch, row-block) pair of the second GAT layer.
    wh_full = h_full @ W_last                        # [N, EN]
    wh_rows = h_rows @ W_last                        # [NR, EN]
    wh1_rows = wh_rows @ a_last[:EN, :]              # [NR, 1]
    wh2_full = wh_full @ a_last[EN:, :]              # [N, 1]
    e = jax.nn.leaky_relu(wh1_rows + wh2_full.T, ALPHA)  # [NR, N]
    nz = adj_rows != 0
    scores = jnp.where(nz, e, NEG_INF) * jnp.where(nz, adj_rows, 1.0)
    att = jax.nn.softmax(scores, axis=-1)
    return jax.nn.elu(att @ wh_full)                 # [NR, EN]


def kernel(x, adj, W, a, W_last, a_last):
    x = jnp.asarray(x, jnp.float32)
    adj = jnp.asarray(adj, jnp.float32)
    W = jnp.asarray(W, jnp.float32)
    a = jnp.asarray(a, jnp.float32)
    W_last = jnp.asarray(W_last, jnp.float32)
    a_last = jnp.asarray(a_last, jnp.float32)

    devs = jax.devices()
    use_pmap = len(devs) >= NCORES

    # ---- Layer 1: shard the [B,H,N,N] attention over the 8 (b,h) pairs ----
    bh = [(b, h) for b in range(B) for h in range(H)]
    x_s = jnp.stack([x[b] for b, h in bh])           # [8, N, D]
    adj_s = jnp.stack([adj[b] for b, h in bh])       # [8, N, N]
    W_s = jnp.stack([W[h] for b, h in bh])           # [8, D, HID]
    a_s = jnp.stack([a[h] for b, h in bh])           # [8, 2*HID, 1]

    if use_pmap:
        out1 = jax.pmap(_layer1_shard, devices=devs[:NCORES])(x_s, adj_s, W_s, a_s)
    else:
        out1 = jax.jit(jax.vmap(_layer1_shard))(x_s, adj_s, W_s, a_s)
    out1 = np.asarray(out1).reshape(B, H, N, HID)

    # ELU + head concat -> [B, N, H*HID]
    h_cat = jax.nn.elu(jnp.asarray(out1)).transpose(0, 2, 1, 3).reshape(B, N, H * HID)
    h_cat = np.asarray(h_cat)

    # ---- Layer 2: shard over (batch, row-block): 2 batches x 4 blocks ----
    blocks = [(b, r) for b in range(B) for r in range(NCORES // B)]
    h_full_s = jnp.stack([jnp.asarray(h_cat[b]) for b, r in blocks])            # [8, N, H*HID]
    h_rows_s = jnp.stack([jnp.asarray(h_cat[b][r * NR:(r + 1) * NR]) for b, r in blocks])
    adj_rows_s = jnp.stack([adj[b, r * NR:(r + 1) * NR] for b, r in blocks])    # [8, NR, N]
    Wl_s = jnp.stack([W_last] * NCORES)
    al_s = jnp.stack([a_last] * NCORES)

    if use_pmap:
        out2 = jax.pmap(_layer2_shard, devices=devs[:NCORES])(
            h_full_s, h_rows_s, adj_rows_s, Wl_s, al_s)
    else:
        out2 = jax.jit(jax.vmap(_layer2_shard))(h_full_s, h_rows_s, adj_rows_s, Wl_s, al_s)
    out2 = np.asarray(out2)                          # [8, NR, EN]

    out = np.empty((B, N, EN), dtype=np.float32)
    for i, (b, r) in enumerate(blocks):
        out[b, r * NR:(r + 1) * NR] = out2[i]
    return out

